# revision 7
# baseline (speedup 1.0000x reference)
"""Chamfer loss kernel for Trainium2 (8 NeuronCores, Bass/Tile).

Problem: pred_points [4, 8192, 3] f32, gt_points [4, 8192, 3] f32 ->
scalar mean(min_j d_ij) + mean(min_i d_ij) over squared pairwise dists.

Strategy
--------
Shard (batch, pred-half): core c handles batch c//2, pred rows
(c%2)*4096 ... +4096, with the full 8192 gt points of that batch.

On each core the [4096, 8192] distance matrix is produced tile-by-tile
on the TensorEngine as ONE augmented K=13 matmul per tile:
    d(i,j) = |x_i|^2 + |y_j|^2 - 2 x_i . y_j
with x = x_hi + x_lo (fp16 hi/lo split; fp16 products accumulate
exactly into fp32 PSUM, so d is accurate to ~1e-6 even though inputs
stream at full fp16 PE rate).

Per [128, 2048] PSUM group, ScalarE casts PSUM fp32 -> SBUF fp16
(frees the Vector engine; 4 groups assemble a full [128, 8192] fp16
row-block d tile). Per row block, the VectorE then runs wide fp16 ops
(2x mode, overheads amortized):
  - one tensor_tensor min against the ping-pong [128, 8192] column-min
    accumulator (dist2 partials)
  - a fold tree 8192->4096->...->256 + one 1x reduce-min -> per-row
    min (dist1 partials)
(the reference's relu clamp commutes with min; the host applies it)

Each core outputs its rowmin partials [128, 32] f32 and colmin
partials [128, 8192] f16; the host does the tiny cross-core/partition
min + mean (exact, ~1MB of numpy work).
"""

import os

import numpy as np

B, N, M, D = 4, 8192, 8192, 3
NCORES = 8
P = 128            # pred rows per block (partition dim)
KAUG = 13          # augmented contraction size
NP = B * N // NCORES   # pred rows per core = 4096
GCOLS = 2048       # gt columns per PSUM group (4 banks)
NMM = 512          # matmul free dim (1 PSUM bank)


def build_nc(NP=NP, M=M, GCOLS=GCOLS, NMM=NMM):
    import concourse.bacc as bacc
    import concourse.mybir as mybir
    import concourse.tile as tile

    f16, f32 = mybir.dt.float16, mybir.dt.float32
    MIN = mybir.AluOpType.min

    nblk = NP // P
    ngrp = M // GCOLS
    nmm = GCOLS // NMM

    nc = bacc.Bacc(target_bir_lowering=False)
    lhs = nc.dram_tensor("lhs_aug", [KAUG, NP], f16, kind="ExternalInput")
    rhs = nc.dram_tensor("rhs_aug", [KAUG, M], f16, kind="ExternalInput")
    colmin_o = nc.dram_tensor("colmin", [P, M], f16, kind="ExternalOutput")
    rowmin_o = nc.dram_tensor("rowmin", [P, nblk], f32, kind="ExternalOutput")

    with tile.TileContext(nc) as tc:
        with (
            tc.tile_pool(name="singles", bufs=1) as singles,
            tc.tile_pool(name="dcopy", bufs=2) as dpool,
            tc.tile_pool(name="scr", bufs=2) as spool,
            tc.tile_pool(name="psum", bufs=2, space="PSUM") as ppool,
        ):
            xw = singles.tile([KAUG, NP], f16)
            yw = singles.tile([KAUG, M], f16)
            nc.sync.dma_start(out=xw[:, :], in_=lhs[:, :])
            nc.sync.dma_start(out=yw[:, :], in_=rhs[:, :])

            colA = singles.tile([P, M], f16)
            colB = singles.tile([P, M], f16)
            rowacc = singles.tile([P, nblk], f32)

            def fold_tree(buf, width, stop, tagp):
                """Pairwise-min fold buf [P, width] down to [P, stop]."""
                prev, w = buf, width // 2
                while w >= stop:
                    f = spool.tile([P, w], f16, tag=f"{tagp}{w}")
                    nc.vector.tensor_tensor(
                        out=f[:, :], in0=prev[:, :w], in1=prev[:, w:2 * w],
                        op=MIN)
                    prev = f
                    w //= 2
                return prev

            rgrp0 = singles.tile([P, ngrp], f32)
            for i in range(nblk):
                src, dst = (colA, colB) if i % 2 == 0 else (colB, colA)
                lhsT = xw[:, i * P:(i + 1) * P]
                # full row-block d tile, assembled group by group
                df = dpool.tile([P, M], f16)
                for g in range(ngrp):
                    ps = ppool.tile([P, GCOLS], f32)
                    for k in range(nmm):
                        nc.tensor.matmul(
                            ps[:, k * NMM:(k + 1) * NMM],
                            lhsT,
                            yw[:, g * GCOLS + k * NMM:g * GCOLS + (k + 1) * NMM],
                            start=True,
                            stop=True,
                        )
                    nc.scalar.copy(
                        df[:, g * GCOLS:(g + 1) * GCOLS], ps[:, :])
                    dfg = df[:, g * GCOLS:(g + 1) * GCOLS]
                    cs = slice(g * GCOLS, (g + 1) * GCOLS)
                    if i == 0:
                        # block 0, per-group: seeds the accumulator with a
                        # plain copy (4x mode, no init memset) and keeps the
                        # pipeline-fill stall short
                        nc.vector.tensor_copy(dst[:, cs], dfg)
                        fg = fold_tree(dfg, GCOLS, 256, "gf")
                        nc.vector.tensor_reduce(
                            out=rgrp0[:, g:g + 1], in_=fg[:, :],
                            axis=mybir.AxisListType.X, op=MIN)
                    elif i == nblk - 1:
                        # last block, per-group: lets the output DMA overlap
                        nc.vector.tensor_tensor(
                            out=dst[:, cs], in0=src[:, cs], in1=dfg, op=MIN)
                        nc.sync.dma_start(out=colmin_o[:, cs], in_=dst[:, cs])
                if i == 0:
                    nc.vector.tensor_reduce(
                        out=rowacc[:, 0:1], in_=rgrp0[:, :],
                        axis=mybir.AxisListType.X, op=MIN)
                    continue
                if i != nblk - 1:
                    # column-min accumulate, whole row block in one op
                    nc.vector.tensor_tensor(
                        out=dst[:, :], in0=src[:, :], in1=df[:, :], op=MIN)
                prev = fold_tree(df, M, 256, "fold")
                nc.vector.tensor_reduce(
                    out=rowacc[:, i:i + 1], in_=prev[:, :],
                    axis=mybir.AxisListType.X, op=MIN,
                )
            # colmin is DMA'd out chunk-by-chunk inside the last block
            nc.sync.dma_start(out=rowmin_o[:, :], in_=rowacc[:, :])
    nc.finalize()  # Bacc defers register allocation to finalize()
    return nc


def _augment(x, y):
    """x [n,3] f32 pred block, y [m,3] f32 gt -> (lhs_aug [13,n] f16,
    rhs_aug [13,m] f16) such that lhs.T @ rhs ~= squared distance matrix."""
    f16, f32 = np.float16, np.float32
    x = np.ascontiguousarray(x, dtype=f32)
    y = np.ascontiguousarray(y, dtype=f32)
    x2 = (x * x).sum(-1)
    y2 = (y * y).sum(-1)
    xh = x.astype(f16)
    xl = (x - xh.astype(f32)).astype(f16)
    yh = y.astype(f16)
    yl = (y - yh.astype(f32)).astype(f16)
    x2h = x2.astype(f16)
    x2l = (x2 - x2h.astype(f32)).astype(f16)
    y2h = y2.astype(f16)
    y2l = (y2 - y2h.astype(f32)).astype(f16)
    m2yh = (yh.astype(f32) * -2.0).astype(f16)   # exact: x2 scaling
    m2yl = (yl.astype(f32) * -2.0).astype(f16)
    n, m = x.shape[0], y.shape[0]
    ones_n = np.ones(n, f16)
    ones_m = np.ones(m, f16)
    lhs = np.stack([
        xh[:, 0], xh[:, 1], xh[:, 2],
        xh[:, 0], xh[:, 1], xh[:, 2],
        xl[:, 0], xl[:, 1], xl[:, 2],
        x2h, x2l, ones_n, ones_n,
    ])  # [13, n]
    rhsa = np.stack([
        m2yh[:, 0], m2yh[:, 1], m2yh[:, 2],
        m2yl[:, 0], m2yl[:, 1], m2yl[:, 2],
        m2yh[:, 0], m2yh[:, 1], m2yh[:, 2],
        ones_m, ones_m, y2h, y2l,
    ])  # [13, m]
    return np.ascontiguousarray(lhs), np.ascontiguousarray(rhsa)


def _make_in_maps(pred_points, gt_points):
    pred = np.asarray(pred_points, dtype=np.float32)
    gt = np.asarray(gt_points, dtype=np.float32)
    in_maps = []
    for c in range(NCORES):
        b, h = c // 2, c % 2
        lhs, rhsa = _augment(pred[b, h * NP:(h + 1) * NP], gt[b])
        in_maps.append({"lhs_aug": lhs, "rhs_aug": rhsa})
    return in_maps


def _finish(results):
    """results: list per core of {'rowmin': [128, nblk] f32,
    'colmin': [128, M] f16} -> scalar chamfer loss."""
    rowsum = np.float64(0.0)
    colsum = np.float64(0.0)
    for c in range(NCORES):
        r = np.maximum(results[c]["rowmin"].astype(np.float64), 0.0)
        rowsum += r.sum()
    for b in range(B):
        m = np.minimum(
            results[2 * b]["colmin"].astype(np.float32),
            results[2 * b + 1]["colmin"].astype(np.float32),
        )
        cm = np.maximum(m.min(axis=0).astype(np.float64), 0.0)
        colsum += cm.sum()
    total = rowsum / (B * N) + colsum / (B * M)
    return np.float32(total)


_RUN_CACHE = {}


def _run_on_hw(in_maps, trace=False, **kw):
    from concourse.bass_utils import run_bass_kernel_spmd

    nc = _RUN_CACHE.get("nc")
    if nc is None:
        nc = build_nc()
        _RUN_CACHE["nc"] = nc
    return run_bass_kernel_spmd(
        nc, in_maps, core_ids=list(range(NCORES)), trace=trace, **kw
    )


def kernel(pred_points, gt_points):
    in_maps = _make_in_maps(pred_points, gt_points)
    br = _run_on_hw(in_maps, trace=False)
    return _finish(br.results)


if __name__ == "__main__":
    pred = np.random.randn(B, N, D).astype(np.float32)
    gt = np.random.randn(B, M, D).astype(np.float32)
    print(kernel(pred, gt))

